# revision 18
# baseline (speedup 1.0000x reference)
"""KIVI attention wrapper — Trainium2 Bass kernel, 8-way head-sharded.

Sharding: 16 heads / 8 cores = 2 heads per core (tensor parallel) for
QKV + attention; token-parallel (512 tokens per core) for c_proj after an
AllToAll of the per-head attention output.

Per core:
- X^T is provided pre-transposed by the host; Q^T/K^T = W^T X^T directly in
  feature-major layout (f32r / tf32 matmuls). V is produced token-major by a
  second GEMM (stationary = X^T blocks). All biases are folded into the
  GEMMs as K=1 accumulation rows (bias x ones outer product).
- KIVI 2-bit fake-quant of K on DVE via 32x32 blockwise stream-transposes
  (group-of-4 absmax along the free axis in the transposed domain).
- Scores computed transposed ([kpos, q]); the two heads' score matmuls are
  row-packed (K=64 at array rows 0-63 / 64-127) and run concurrently into
  one [128, 1024] PSUM region; one exp per region; AV software-pipelined
  one region behind the score matmuls.
- The additive attention mask is folded into the V tiles (and the softmax
  denominator column) as exp(mask/8) row scaling — exact.
- Softmax normalization deferred: unnormalized O~ plus the per-(head,q)
  denominator rows travel through the AllToAll; one [16, 512] reciprocal
  per core afterwards.
- c_proj token-sharded: each core computes all 1024 output columns for its
  512 tokens; output returned column-major [1024, 512] per core.
- Program order software-pipelines chunks: attention for batch b is
  emitted between the QKV/quant stages of later chunks so the PE never
  waits on the DVE quant chain (engine queues are FIFO).
"""
import sys
sys.path.insert(0, '/opt/trn_rl_repo')
import numpy as np

P = 128
TOK = 4096          # B*S = 4*1024
E = 1024
NB = 8              # embed 128-blocks
CH = 512            # token chunk
NCH = 8             # token 512-chunks
MAGIC = 8388608.0   # 2^23: z + MAGIC - MAGIC == rint(z) for 0 <= z < 2^22
HSH = 65 * CH       # per-head shard half: 64 o rows + 1 denominator row
OSH = 2 * HSH       # flat a2a shard

_CACHE = {}


def _build(sim_single=False):
    import concourse.bacc as bacc
    import concourse.mybir as mybir
    import concourse.tile as tile

    f32 = mybir.dt.float32
    fmm = mybir.dt.float32r
    X = mybir.AxisListType.X
    ADD = mybir.AluOpType.add
    MULT = mybir.AluOpType.mult
    MAX = mybir.AluOpType.max
    SUB = mybir.AluOpType.subtract
    EXP = mybir.ActivationFunctionType.Exp
    IDENT = mybir.ActivationFunctionType.Identity

    nc = bacc.Bacc("TRN2", target_bir_lowering=False, debug=False,
                   num_devices=(1 if sim_single else 8))

    xt_ap = nc.dram_tensor("xt", [E, TOK], fmm, kind="ExternalInput").ap()
    wqkv_ap = nc.dram_tensor("wqkv", [E, 384], fmm, kind="ExternalInput").ap()
    brow_ap = nc.dram_tensor("brow", [1, 384], fmm, kind="ExternalInput").ap()
    m8t_ap = nc.dram_tensor("m8t", [P, 32], f32, kind="ExternalInput").ap()
    wp_ap = nc.dram_tensor("wp", [E, E], fmm, kind="ExternalInput").ap()
    bpt_ap = nc.dram_tensor("bpt", [P, 8], f32, kind="ExternalInput").ap()
    e16_ap = nc.dram_tensor("e16", [16, E], fmm, kind="ExternalInput").ap()
    yt_ap = nc.dram_tensor("yt", [E, CH], f32, kind="ExternalOutput").ap()

    with tile.TileContext(nc) as tc:
        with tc.tile_pool(name="const", bufs=1) as constp, \
             tc.tile_pool(name="big", bufs=1) as bigp, \
             tc.tile_pool(name="xbp", bufs=12) as xbp, \
             tc.tile_pool(name="s1w", bufs=2) as s1w, \
             tc.tile_pool(name="qw", bufs=2) as qw, \
             tc.tile_pool(name="esp", bufs=2) as esp, \
             tc.tile_pool(name="otp", bufs=2) as otp, \
             tc.tile_pool(name="s5o", bufs=1) as s5o, \
             tc.tile_pool(name="s5p", bufs=2) as s5p, \
             tc.tile_pool(name="pA", bufs=2, space="PSUM") as pA, \
             tc.tile_pool(name="pB", bufs=2, space="PSUM") as pB, \
             tc.tile_pool(name="pC", bufs=1, space="PSUM") as pC, \
             tc.tile_pool(name="dram", bufs=1, space="DRAM") as dramp:

            browt = constp.tile([1, 384], fmm)
            nc.sync.dma_start(browt[:], brow_ap)
            bptt = constp.tile([P, 8], f32)
            nc.sync.dma_start(bptt[:], bpt_ap)
            e16t = constp.tile([16, E], fmm)
            nc.sync.dma_start(e16t[:], e16_ap)
            m8tt = constp.tile([P, 32], f32)
            nc.sync.dma_start(m8tt[:], m8t_ap)
            emask = constp.tile([P, 32], fmm, name="emask", tag="emask")
            nc.scalar.activation(emask[:], m8tt[:], EXP)
            ones32 = constp.tile([1, CH], f32, name="ones32", tag="ones32")
            nc.any.memset(ones32[:], 1.0)
            onesr = constp.tile([1, CH], fmm, name="onesr", tag="onesr")
            nc.vector.tensor_copy(onesr[:], ones32[:])
            wts = []
            for eb in range(NB):
                wt = constp.tile([P, 384], fmm, name=f"wt{eb}", tag=f"wt{eb}")
                nc.sync.dma_start(wt[:], wqkv_ap[eb * P:(eb + 1) * P, :])
                wts.append(wt)
            wps = []
            for eb in range(NB):
                wpt = constp.tile([P, E], fmm, name=f"wp{eb}", tag=f"wp{eb}")
                nc.sync.dma_start(wpt[:], wp_ap[eb * P:(eb + 1) * P, :])
                wps.append(wpt)

            # persistent feature-major tensors [128 = 2 heads x 64, 4096 tok]
            qT = bigp.tile([P, TOK], fmm, tag="qT")
            kdT = bigp.tile([P, TOK], fmm, tag="kdT")
            vtiles = []   # [gkb][h] -> [128 kpos, 65] (64 feats + emask col)

            agin = dramp.tile([NCH, OSH], fmm, tag="agin")
            agout = dramp.tile([NCH, OSH], fmm, tag="agout")

            def s12(ch):
                t0 = ch * CH
                # ---- S1: Q^T/K^T GEMM (feature-major, bias as K=1 row) ----
                xbs = []
                for eb in range(NB):
                    xb = xbp.tile([P, CH], fmm, name=f"xb{ch}_{eb}", tag="xb")
                    nc.sync.dma_start(
                        xb[:], xt_ap[eb * P:(eb + 1) * P, t0:t0 + CH])
                    xbs.append(xb)
                kc = s1w.tile([P, CH], f32, tag="kc")
                for m in range(2):
                    gps = pA.tile([P, CH], f32, tag="pa")
                    for eb in range(NB):
                        nc.tensor.matmul(
                            gps[:], wts[eb][:, m * P:(m + 1) * P], xbs[eb][:],
                            start=(eb == 0), stop=(eb == NB - 1))
                    if m == 0:
                        nc.vector.tensor_copy(qT[:, t0:t0 + CH], gps[:])
                    else:
                        nc.vector.tensor_copy(kc[:], gps[:])

                # ---- S2a: KIVI fake-quant of K (DVE + GpSimd) ----
                kq = qw.tile([P, CH], f32, tag="kq")
                nc.vector.transpose(kq[:], kc[:])
                gmax = qw.tile([P, P], f32, tag="gmax")
                nc.vector.tensor_reduce(
                    gmax[:], kq[:].rearrange("p (g f) -> p g f", f=4),
                    axis=X, op=MAX, apply_absolute_value=True)
                rs = qw.tile([P, P], f32, tag="rs")
                nc.vector.reciprocal(rs[:], gmax[:])
                kd = qw.tile([P, CH], f32, tag="kd")
                kd_g = kd[:].rearrange("p (g f) -> p g f", f=4)
                kq_g = kq[:].rearrange("p (g f) -> p g f", f=4)
                nc.gpsimd.tensor_tensor(
                    kd_g, kq_g, rs[:, :, None].to_broadcast((P, P, 4)), MULT)
                # codes = rint(1.5*kd + 1.5) in [0,3]; then codes - 1.5
                nc.vector.tensor_scalar(kd[:], kd[:], 1.5, 1.5, MULT, ADD)
                nc.vector.tensor_scalar(kd[:], kd[:], MAGIC, MAGIC, ADD, SUB)
                nc.vector.tensor_scalar_sub(kd[:], kd[:], 1.5)
                # dequant*1.5: (codes-1.5)*absmax  (1/1.5 folded into W_q)
                kdq = qw.tile([P, CH], f32, tag="kdq")
                nc.gpsimd.tensor_tensor(
                    kdq[:].rearrange("p (g f) -> p g f", f=4), kd_g,
                    gmax[:, :, None].to_broadcast((P, P, 4)), MULT)
                kdn = qw.tile([P, CH], f32, tag="kdn")
                nc.vector.transpose(kdn[:], kdq[:])
                nc.scalar.copy(kdT[:, t0:t0 + CH], kdn[:])

                # ---- S2b: V token-major GEMM + mask-scaled V tiles ----
                for tb in range(4):
                    gkb = ch * 4 + tb
                    psv = pA.tile([P, P], f32, tag="pa")
                    for eb in range(NB):
                        nc.tensor.matmul(
                            psv[:], xbs[eb][:, tb * P:(tb + 1) * P],
                            wts[eb][:, 256:384],
                            start=(eb == 0), stop=(eb == NB - 1))
                    vh = []
                    for h in range(2):
                        v = bigp.tile([P, 65], fmm, name=f"v{gkb}_{h}",
                                      tag=f"v{gkb}_{h}")
                        nc.vector.tensor_tensor(
                            v[:, 0:64], psv[:, h * 64:(h + 1) * 64],
                            emask[:, gkb:gkb + 1].to_broadcast((P, 64)), MULT)
                        nc.vector.tensor_copy(
                            v[:, 64:65], emask[:, gkb:gkb + 1])
                        vh.append(v)
                    vtiles.append(vh)

            def s3(b):
                for qc in range(2):
                    j = b * 2 + qc
                    q0 = j * CH
                    cav = [pC.tile([65, CH], f32, name=f"cav{j}_{h}",
                                   tag=f"cav{h}")
                           for h in range(2)]
                    es_list = []

                    def av(kb):
                        gkb = b * 8 + kb
                        for h in range(2):
                            nc.tensor.matmul(
                                cav[h][:], vtiles[gkb][h][:],
                                es_list[kb][:, h * CH:(h + 1) * CH],
                                start=(kb == 0), stop=(kb == 7),
                                skip_group_check=True)

                    for kb in range(8):
                        gkb = b * 8 + kb
                        ps_s = pB.tile([P, 2 * CH], f32, tag="pb")
                        for h in range(2):
                            nc.tensor.matmul(
                                ps_s[:, h * CH:(h + 1) * CH],
                                kdT[h * 64:(h + 1) * 64,
                                    gkb * P:(gkb + 1) * P],
                                qT[h * 64:(h + 1) * 64, q0:q0 + CH],
                                start=True, stop=True)
                        es = esp.tile([P, 2 * CH], fmm, tag="es")
                        nc.scalar.activation(es[:], ps_s[:], EXP, scale=0.125)
                        es_list.append(es)
                        if kb >= 1:
                            av(kb - 1)
                    av(7)
                    for h in range(2):
                        o65 = otp.tile([65, CH], fmm, name=f"o65{j}_{h}",
                                       tag=f"o65{h}")
                        nc.vector.tensor_copy(o65[:], cav[h][:])
                        nc.sync.dma_start(
                            agin[j, h * HSH:(h + 1) * HSH]
                            .rearrange("(p f) -> p f", p=65), o65[:])

            # software-pipelined program order: attention for batch b sits
            # between the QKV/quant of later chunks so PE never waits on DVE
            s12(0); s12(1); s12(2)
            s3(0)
            s12(3); s12(4)
            s3(1)
            s12(5); s12(6)
            s3(2)
            s12(7)
            s3(3)

            # ---------------- S4: AllToAll ---------------------------
            if sim_single:
                nc.gpsimd.dma_start(agout[:], agin[:])
            else:
                nc.gpsimd.collective_compute(
                    "AllToAll", mybir.AluOpType.bypass,
                    replica_groups=[list(range(8))],
                    ins=[agin[:]], outs=[agout[:]])

            # ---------------- S5: normalize + c_proj (my 512 tokens) --
            rcpin = constp.tile([16, CH], fmm, tag="rcpin")
            nc.sync.dma_start(rcpin[0:8, :], agout[:, 64 * CH:HSH])
            nc.sync.dma_start(rcpin[8:16, :], agout[:, HSH + 64 * CH:OSH])
            rcpt = constp.tile([16, CH], fmm, tag="rcpt")
            with nc.allow_low_precision(reason="tf32 recip"):
                nc.vector.reciprocal(rcpt[:], rcpin[:])
            ogs = []
            for fb in range(NB):
                og = s5o.tile([P, CH], fmm, name=f"og{fb}", tag=f"og{fb}")
                nc.sync.dma_start(
                    og[0:64, :],
                    agout[fb, 0:64 * CH].rearrange("(p f) -> p f", p=64))
                nc.sync.dma_start(
                    og[64:128, :],
                    agout[fb, HSH:HSH + 64 * CH]
                    .rearrange("(p f) -> p f", p=64))
                ps_r = pA.tile([P, CH], f32, tag="pa")
                nc.tensor.matmul(ps_r[:], e16t[:, fb * P:(fb + 1) * P],
                                 rcpt[:], start=True, stop=True)
                rrep = s5p.tile([P, CH], f32, tag="rrep")
                nc.scalar.copy(rrep[:], ps_r[:])
                nc.vector.tensor_tensor(og[:], og[:], rrep[:], MULT)
                ogs.append(og)
            for ob in range(NB):
                ps_p = pA.tile([P, CH], f32, tag="pa")
                for fb in range(NB):
                    nc.tensor.matmul(
                        ps_p[:], wps[fb][:, ob * P:(ob + 1) * P], ogs[fb][:],
                        start=(fb == 0), stop=(fb == NB - 1))
                yts = s5p.tile([P, CH], f32, tag="yts")
                nc.vector.tensor_tensor(
                    yts[:], ps_p[:],
                    bptt[:, ob:ob + 1].to_broadcast((P, CH)), ADD)
                nc.sync.dma_start(yt_ap[ob * P:(ob + 1) * P, :], yts[:])

    nc.compile()
    return nc


def make_in_maps(hidden_states, attention_mask, w_attn, b_attn, w_proj, b_proj):
    x = np.asarray(hidden_states, np.float32).reshape(TOK, E)
    xt = np.ascontiguousarray(x.T)
    mask = np.asarray(attention_mask, np.float32)
    wa = np.asarray(w_attn, np.float32)
    ba = np.asarray(b_attn, np.float32)
    wpf = np.ascontiguousarray(np.asarray(w_proj, np.float32))
    bp = np.asarray(b_proj, np.float32)

    m8 = (mask * np.float32(0.125)).reshape(4, 8, 128)
    m8t = np.ascontiguousarray(m8.transpose(2, 0, 1).reshape(128, 32))
    bpt = np.ascontiguousarray(bp.reshape(8, P).T)
    e16 = np.zeros((16, E), dtype=np.float32)
    for r in range(16):
        h = 2 * r if r < 8 else 2 * (r - 8) + 1
        fb, half = h // 2, h % 2
        e16[r, fb * P + half * 64: fb * P + half * 64 + 64] = 1.0

    in_maps = []
    for c in range(8):
        cs = slice(c * P, (c + 1) * P)
        wqkv = np.concatenate(
            [wa[:, cs] * np.float32(2.0 / 3.0),
             wa[:, 1024 + c * P:1024 + (c + 1) * P],
             wa[:, 2048 + c * P:2048 + (c + 1) * P]], axis=1)
        brow = np.concatenate(
            [ba[cs] * np.float32(2.0 / 3.0),
             ba[1024 + c * P:1024 + (c + 1) * P],
             ba[2048 + c * P:2048 + (c + 1) * P]])[None, :]
        in_maps.append({
            "xt": xt, "wqkv": np.ascontiguousarray(wqkv),
            "brow": np.ascontiguousarray(brow), "m8t": m8t,
            "wp": wpf, "bpt": bpt, "e16": e16,
        })
    return in_maps


def kernel(hidden_states, attention_mask, w_attn, b_attn, w_proj, b_proj):
    from concourse import bass_utils
    if "nc" not in _CACHE:
        _CACHE["nc"] = _build()
    nc = _CACHE["nc"]
    in_maps = make_in_maps(hidden_states, attention_mask, w_attn, b_attn,
                           w_proj, b_proj)
    res = bass_utils.run_bass_kernel_spmd(nc, in_maps, core_ids=list(range(8)))
    y = np.empty((TOK, E), dtype=np.float32)
    for c in range(8):
        y[c * CH:(c + 1) * CH, :] = res.results[c]["yt"].T
    return y.reshape(4, 1024, E)


# revision 24
# speedup vs baseline: 207.2773x; 207.2773x over previous
"""KIVI attention wrapper — Trainium2 Bass kernel, 8-way head-sharded.

Sharding: 16 heads / 8 cores = 2 heads per core (tensor parallel) for
QKV + attention; token-parallel (512 tokens per core) for c_proj after an
AllToAll of the per-head attention output.

Per core:
- X^T is provided pre-transposed by the host; Q^T/K^T = W^T X^T directly in
  feature-major layout (f32r / tf32 matmuls). V is produced token-major by a
  second GEMM (stationary = X^T blocks). All biases are folded into the
  GEMMs as K=1 accumulation rows (bias x ones outer product).
- KIVI 2-bit fake-quant of K on DVE via 32x32 blockwise stream-transposes
  (group-of-4 absmax along the free axis in the transposed domain).
- Scores computed transposed ([kpos, q]); the two heads' score matmuls are
  row-packed (K=64 at array rows 0-63 / 64-127) and run concurrently into
  one [128, 1024] PSUM region; one exp per region; AV software-pipelined
  one region behind the score matmuls.
- The additive attention mask is folded into the V tiles (and the softmax
  denominator column) as exp(mask/8) row scaling — exact.
- Softmax normalization deferred: unnormalized O~ plus the per-(head,q)
  denominator rows travel through the AllToAll; one [16, 512] reciprocal
  per core afterwards.
- c_proj token-sharded: each core computes all 1024 output columns for its
  512 tokens; output returned column-major [1024, 512] per core.
- Program order software-pipelines chunks: attention for batch b is
  emitted between the QKV/quant stages of later chunks so the PE never
  waits on the DVE quant chain (engine queues are FIFO).
"""
import sys
sys.path.insert(0, '/opt/trn_rl_repo')
import numpy as np

P = 128
TOK = 4096          # B*S = 4*1024
E = 1024
NB = 8              # embed 128-blocks
CH = 512            # token chunk
NCH = 8             # token 512-chunks
MAGIC = 8388608.0   # 2^23: z + MAGIC - MAGIC == rint(z) for 0 <= z < 2^22
HSH = 65 * CH       # per-head shard half: 64 o rows + 1 denominator row
OSH = 2 * HSH       # flat a2a shard

_CACHE = {}


def _build(sim_single=False):
    import concourse.bacc as bacc
    import concourse.mybir as mybir
    import concourse.tile as tile

    f32 = mybir.dt.float32
    fmm = mybir.dt.float32r
    X = mybir.AxisListType.X
    ADD = mybir.AluOpType.add
    MULT = mybir.AluOpType.mult
    MAX = mybir.AluOpType.max
    SUB = mybir.AluOpType.subtract
    EXP = mybir.ActivationFunctionType.Exp
    IDENT = mybir.ActivationFunctionType.Identity

    nc = bacc.Bacc("TRN2", target_bir_lowering=False, debug=False,
                   num_devices=(1 if sim_single else 8))

    xt_ap = nc.dram_tensor("xt", [E, TOK], fmm, kind="ExternalInput").ap()
    wqkv_ap = nc.dram_tensor("wqkv", [E, 384], fmm, kind="ExternalInput").ap()
    bqk_ap = nc.dram_tensor("bqk", [P, 2], f32, kind="ExternalInput").ap()
    bvg_ap = nc.dram_tensor("bvg", [P, 8], f32, kind="ExternalInput").ap()
    m8t_ap = nc.dram_tensor("m8t", [P, 32], f32, kind="ExternalInput").ap()
    wp_ap = nc.dram_tensor("wp", [E, E], fmm, kind="ExternalInput").ap()
    bpt_ap = nc.dram_tensor("bpt", [P, 8], f32, kind="ExternalInput").ap()
    e16_ap = nc.dram_tensor("e16", [16, E], fmm, kind="ExternalInput").ap()
    yt_ap = nc.dram_tensor("yt", [E, CH], f32, kind="ExternalOutput").ap()

    with tile.TileContext(nc) as tc:
        with tc.tile_pool(name="const", bufs=1) as constp, \
             tc.tile_pool(name="big", bufs=1) as bigp, \
             tc.tile_pool(name="xbp", bufs=12) as xbp, \
             tc.tile_pool(name="s1w", bufs=2) as s1w, \
             tc.tile_pool(name="qw", bufs=2) as qw, \
             tc.tile_pool(name="esp", bufs=2) as esp, \
             tc.tile_pool(name="otp", bufs=2) as otp, \
             tc.tile_pool(name="s5o", bufs=1) as s5o, \
             tc.tile_pool(name="s5p", bufs=2) as s5p, \
             tc.tile_pool(name="pA", bufs=2, space="PSUM") as pA, \
             tc.tile_pool(name="pB", bufs=2, space="PSUM") as pB, \
             tc.tile_pool(name="pC", bufs=1, space="PSUM") as pC, \
             tc.tile_pool(name="dram", bufs=1, space="DRAM") as dramp:

            bqkt = constp.tile([P, 2], f32)
            nc.sync.dma_start(bqkt[:], bqk_ap)
            bvgt = constp.tile([P, 8], f32)
            nc.sync.dma_start(bvgt[:], bvg_ap)
            bptt = constp.tile([P, 8], f32)
            nc.sync.dma_start(bptt[:], bpt_ap)
            e16t = constp.tile([16, E], fmm)
            nc.sync.dma_start(e16t[:], e16_ap)
            m8tt = constp.tile([P, 32], f32)
            nc.sync.dma_start(m8tt[:], m8t_ap)
            emask = constp.tile([P, 32], fmm, name="emask", tag="emask")
            nc.scalar.activation(emask[:], m8tt[:], EXP)
            wts = []
            for eb in range(NB):
                wt = constp.tile([P, 384], fmm, name=f"wt{eb}", tag=f"wt{eb}")
                nc.sync.dma_start(wt[:], wqkv_ap[eb * P:(eb + 1) * P, :])
                wts.append(wt)
            wps = []
            for eb in range(NB):
                wpt = constp.tile([P, E], fmm, name=f"wp{eb}", tag=f"wp{eb}")
                nc.sync.dma_start(wpt[:], wp_ap[eb * P:(eb + 1) * P, :])
                wps.append(wpt)

            # persistent feature-major tensors [128 = 2 heads x 64, 4096 tok]
            qT = bigp.tile([P, TOK], fmm, tag="qT")
            kdT = bigp.tile([P, TOK], fmm, tag="kdT")
            vtiles = []   # [gkb][h] -> [128 kpos, 65] (64 feats + emask col)

            agin = dramp.tile([NCH, OSH], fmm, tag="agin")
            agout = dramp.tile([NCH, OSH], fmm, tag="agout")

            def s12(ch):
                t0 = ch * CH
                # ---- S1: Q^T/K^T GEMM (feature-major, bias as K=1 row) ----
                xbs = []
                for eb in range(NB):
                    xb = xbp.tile([P, CH], fmm, name=f"xb{ch}_{eb}", tag="xb")
                    nc.sync.dma_start(
                        xb[:], xt_ap[eb * P:(eb + 1) * P, t0:t0 + CH])
                    xbs.append(xb)
                kc = s1w.tile([P, CH], f32, tag="kc")
                for m in range(2):
                    gps = pA.tile([P, CH], f32, tag="pa")
                    for eb in range(NB):
                        nc.tensor.matmul(
                            gps[:], wts[eb][:, m * P:(m + 1) * P], xbs[eb][:],
                            start=(eb == 0), stop=(eb == NB - 1))
                    dst = qT[:, t0:t0 + CH] if m == 0 else kc[:]
                    nc.vector.tensor_tensor(
                        dst, gps[:],
                        bqkt[:, m:m + 1].to_broadcast((P, CH)), ADD)

                # ---- S2a: KIVI fake-quant of K (DVE + GpSimd) ----
                kq = qw.tile([P, CH], f32, tag="kq")
                nc.vector.transpose(kq[:], kc[:])
                gmax = qw.tile([P, P], f32, tag="gmax")
                nc.vector.tensor_reduce(
                    gmax[:], kq[:].rearrange("p (g f) -> p g f", f=4),
                    axis=X, op=MAX, apply_absolute_value=True)
                rs = qw.tile([P, P], f32, tag="rs")
                nc.vector.reciprocal(rs[:], gmax[:])
                kd = qw.tile([P, CH], f32, tag="kd")
                kd_g = kd[:].rearrange("p (g f) -> p g f", f=4)
                kq_g = kq[:].rearrange("p (g f) -> p g f", f=4)
                nc.gpsimd.tensor_tensor(
                    kd_g, kq_g, rs[:, :, None].to_broadcast((P, P, 4)), MULT)
                # codes = rint(1.5*kd + 1.5) in [0,3]; then codes - 1.5
                nc.vector.tensor_scalar(kd[:], kd[:], 1.5, 1.5, MULT, ADD)
                nc.vector.tensor_scalar(kd[:], kd[:], MAGIC, MAGIC, ADD, SUB)
                nc.vector.tensor_scalar_sub(kd[:], kd[:], 1.5)
                # dequant*1.5: (codes-1.5)*absmax  (1/1.5 folded into W_q)
                kdq = qw.tile([P, CH], f32, tag="kdq")
                nc.gpsimd.tensor_tensor(
                    kdq[:].rearrange("p (g f) -> p g f", f=4), kd_g,
                    gmax[:, :, None].to_broadcast((P, P, 4)), MULT)
                kdn = qw.tile([P, CH], f32, tag="kdn")
                nc.vector.transpose(kdn[:], kdq[:])
                nc.scalar.copy(kdT[:, t0:t0 + CH], kdn[:])

                # ---- S2b: V token-major GEMM + mask-scaled V tiles ----
                for tb in range(4):
                    gkb = ch * 4 + tb
                    psv = pA.tile([P, P], f32, tag="pa")
                    for eb in range(NB):
                        nc.tensor.matmul(
                            psv[:], xbs[eb][:, tb * P:(tb + 1) * P],
                            wts[eb][:, 256:384],
                            start=(eb == 0), stop=(eb == NB - 1))
                    vh = []
                    for h in range(2):
                        v = bigp.tile([P, 65], fmm, name=f"v{gkb}_{h}",
                                      tag=f"v{gkb}_{h}")
                        nc.vector.tensor_tensor(
                            v[:, 0:64], psv[:, h * 64:(h + 1) * 64],
                            emask[:, gkb:gkb + 1].to_broadcast((P, 64)), MULT)
                        nc.vector.tensor_copy(
                            v[:, 64:65], emask[:, gkb:gkb + 1])
                        vh.append(v)
                    vtiles.append(vh)

            def s3(b):
                for qc in range(2):
                    j = b * 2 + qc
                    q0 = j * CH
                    cav = [pC.tile([65, CH], f32, name=f"cav{j}_{h}",
                                   tag=f"cav{h}")
                           for h in range(2)]
                    es_list = []

                    def av(kb):
                        gkb = b * 8 + kb
                        for h in range(2):
                            nc.tensor.matmul(
                                cav[h][:], vtiles[gkb][h][:],
                                es_list[kb][:, h * CH:(h + 1) * CH],
                                start=(kb == 0), stop=(kb == 7),
                                skip_group_check=True)

                    for kb in range(8):
                        gkb = b * 8 + kb
                        ps_s = pB.tile([P, 2 * CH], f32, tag="pb")
                        for h in range(2):
                            nc.tensor.matmul(
                                ps_s[:, h * CH:(h + 1) * CH],
                                kdT[h * 64:(h + 1) * 64,
                                    gkb * P:(gkb + 1) * P],
                                qT[h * 64:(h + 1) * 64, q0:q0 + CH],
                                start=True, stop=True)
                        es = esp.tile([P, 2 * CH], fmm, tag="es")
                        nc.scalar.activation(es[:], ps_s[:], EXP, scale=0.125)
                        es_list.append(es)
                        if kb >= 1:
                            av(kb - 1)
                    av(7)
                    for h in range(2):
                        o65 = otp.tile([65, CH], fmm, name=f"o65{j}_{h}",
                                       tag=f"o65{h}")
                        nc.vector.tensor_copy(o65[:], cav[h][:])
                        nc.sync.dma_start(
                            agin[j, h * HSH:(h + 1) * HSH]
                            .rearrange("(p f) -> p f", p=65), o65[:])

            # software-pipelined program order: attention for batch b sits
            # between the QKV/quant of later chunks so PE never waits on DVE
            s12(0); s12(1); s12(2)
            s3(0)
            s12(3); s12(4)
            s3(1)
            s12(5); s12(6)
            s3(2)
            s12(7)
            s3(3)

            # ---------------- S4: AllToAll ---------------------------
            if sim_single:
                nc.gpsimd.dma_start(agout[:], agin[:])
            else:
                nc.gpsimd.collective_compute(
                    "AllToAll", mybir.AluOpType.bypass,
                    replica_groups=[list(range(8))],
                    ins=[agin[:]], outs=[agout[:]])

            # ---------------- S5: normalize + c_proj (my 512 tokens) --
            rcpin = constp.tile([16, CH], fmm, tag="rcpin")
            nc.sync.dma_start(rcpin[0:8, :], agout[:, 64 * CH:HSH])
            nc.sync.dma_start(rcpin[8:16, :], agout[:, HSH + 64 * CH:OSH])
            rcpt = constp.tile([16, CH], fmm, tag="rcpt")
            with nc.allow_low_precision(reason="tf32 recip"):
                nc.vector.reciprocal(rcpt[:], rcpin[:])
            ogs = []
            for fb in range(NB):
                og = s5o.tile([P, CH], fmm, name=f"og{fb}", tag=f"og{fb}")
                nc.sync.dma_start(
                    og[0:64, :],
                    agout[fb, 0:64 * CH].rearrange("(p f) -> p f", p=64))
                nc.sync.dma_start(
                    og[64:128, :],
                    agout[fb, HSH:HSH + 64 * CH]
                    .rearrange("(p f) -> p f", p=64))
                ps_r = pA.tile([P, CH], f32, tag="pa")
                nc.tensor.matmul(ps_r[:], e16t[:, fb * P:(fb + 1) * P],
                                 rcpt[:], start=True, stop=True)
                rrep = s5p.tile([P, CH], f32, tag="rrep")
                nc.scalar.copy(rrep[:], ps_r[:])
                nc.vector.tensor_tensor(og[:], og[:], rrep[:], MULT)
                # deferred V bias: attn @ (V + 1 bv^T) == o~ + den bv^T, so
                # after normalization just add bv per feature row
                nc.vector.tensor_scalar_add(og[:], og[:], bvgt[:, fb:fb + 1])
                ogs.append(og)
            for ob in range(NB):
                ps_p = pA.tile([P, CH], f32, tag="pa")
                for fb in range(NB):
                    nc.tensor.matmul(
                        ps_p[:], wps[fb][:, ob * P:(ob + 1) * P], ogs[fb][:],
                        start=(fb == 0), stop=(fb == NB - 1))
                yts = s5p.tile([P, CH], f32, tag="yts")
                nc.vector.tensor_tensor(
                    yts[:], ps_p[:],
                    bptt[:, ob:ob + 1].to_broadcast((P, CH)), ADD)
                nc.sync.dma_start(yt_ap[ob * P:(ob + 1) * P, :], yts[:])

    nc.compile()
    return nc


def make_in_maps(hidden_states, attention_mask, w_attn, b_attn, w_proj, b_proj):
    x = np.asarray(hidden_states, np.float32).reshape(TOK, E)
    xt = np.ascontiguousarray(x.T)
    mask = np.asarray(attention_mask, np.float32)
    wa = np.asarray(w_attn, np.float32)
    ba = np.asarray(b_attn, np.float32)
    wpf = np.ascontiguousarray(np.asarray(w_proj, np.float32))
    bp = np.asarray(b_proj, np.float32)

    m8 = (mask * np.float32(0.125)).reshape(4, 8, 128)
    m8t = np.ascontiguousarray(m8.transpose(2, 0, 1).reshape(128, 32))
    bpt = np.ascontiguousarray(bp.reshape(8, P).T)
    e16 = np.zeros((16, E), dtype=np.float32)
    for r in range(16):
        h = 2 * r if r < 8 else 2 * (r - 8) + 1
        fb, half = h // 2, h % 2
        e16[r, fb * P + half * 64: fb * P + half * 64 + 64] = 1.0

    in_maps = []
    for c in range(8):
        cs = slice(c * P, (c + 1) * P)
        wqkv = np.concatenate(
            [wa[:, cs] * np.float32(2.0 / 3.0),
             wa[:, 1024 + c * P:1024 + (c + 1) * P],
             wa[:, 2048 + c * P:2048 + (c + 1) * P]], axis=1)
        bqk = np.stack(
            [ba[cs] * np.float32(2.0 / 3.0),
             ba[1024 + c * P:1024 + (c + 1) * P]], axis=1)
        bvg = np.ascontiguousarray(ba[2048:3072].reshape(8, P).T)
        in_maps.append({
            "xt": xt, "wqkv": np.ascontiguousarray(wqkv),
            "bqk": np.ascontiguousarray(bqk), "bvg": bvg, "m8t": m8t,
            "wp": wpf, "bpt": bpt, "e16": e16,
        })
    return in_maps


def kernel(hidden_states, attention_mask, w_attn, b_attn, w_proj, b_proj):
    from concourse import bass_utils
    if "nc" not in _CACHE:
        _CACHE["nc"] = _build()
    nc = _CACHE["nc"]
    in_maps = make_in_maps(hidden_states, attention_mask, w_attn, b_attn,
                           w_proj, b_proj)
    res = bass_utils.run_bass_kernel_spmd(nc, in_maps, core_ids=list(range(8)))
    y = np.empty((TOK, E), dtype=np.float32)
    for c in range(8):
        y[c * CH:(c + 1) * CH, :] = res.results[c]["yt"].T
    return y.reshape(4, 1024, E)


# revision 31
# speedup vs baseline: 253.7910x; 1.2244x over previous
"""KIVI attention wrapper — Trainium2 Bass kernel, 8-way head-sharded.

Sharding: 16 heads / 8 cores = 2 heads per core (tensor parallel) for
QKV + attention; token-parallel (512 tokens per core) for c_proj after an
AllToAll of the per-head attention output.

Per core:
- X^T is provided pre-transposed by the host; Q^T/K^T = W^T X^T directly in
  feature-major layout (f32r / tf32 matmuls). V is produced token-major by a
  second GEMM (stationary = X^T blocks). All biases are folded into the
  GEMMs as K=1 accumulation rows (bias x ones outer product).
- KIVI 2-bit fake-quant of K on DVE via 32x32 blockwise stream-transposes
  (group-of-4 absmax along the free axis in the transposed domain).
- Scores computed transposed ([kpos, q]); the two heads' score matmuls are
  row-packed (K=64 at array rows 0-63 / 64-127) and run concurrently into
  one [128, 1024] PSUM region; one exp per region; AV software-pipelined
  one region behind the score matmuls.
- The additive attention mask is folded into the V tiles (and the softmax
  denominator column) as exp(mask/8) row scaling — exact.
- Softmax normalization deferred: unnormalized O~ plus the per-(head,q)
  denominator rows travel through the AllToAll; one [16, 512] reciprocal
  per core afterwards.
- c_proj token-sharded: each core computes all 1024 output columns for its
  512 tokens; output returned column-major [1024, 512] per core.
- Program order software-pipelines chunks: attention for batch b is
  emitted between the QKV/quant stages of later chunks so the PE never
  waits on the DVE quant chain (engine queues are FIFO).
"""
import sys
sys.path.insert(0, '/opt/trn_rl_repo')
import numpy as np

P = 128
TOK = 4096          # B*S = 4*1024
E = 1024
NB = 8              # embed 128-blocks
CH = 512            # token chunk
NCH = 8             # token 512-chunks
MAGIC = 8388608.0   # 2^23: z + MAGIC - MAGIC == rint(z) for 0 <= z < 2^22
HSH = 65 * CH       # per-head shard half: 64 o rows + 1 denominator row
OSH = 2 * HSH       # flat a2a shard

_CACHE = {}


def _build(sim_single=False):
    import concourse.bacc as bacc
    import concourse.mybir as mybir
    import concourse.tile as tile

    f32 = mybir.dt.float32
    fmm = mybir.dt.float32r
    X = mybir.AxisListType.X
    ADD = mybir.AluOpType.add
    MULT = mybir.AluOpType.mult
    MAX = mybir.AluOpType.max
    SUB = mybir.AluOpType.subtract
    EXP = mybir.ActivationFunctionType.Exp
    IDENT = mybir.ActivationFunctionType.Identity

    nc = bacc.Bacc("TRN2", target_bir_lowering=False, debug=False,
                   num_devices=(1 if sim_single else 8))

    xt_ap = nc.dram_tensor("xt", [E, TOK], fmm, kind="ExternalInput").ap()
    wqkv_ap = nc.dram_tensor("wqkv", [E, 384], fmm, kind="ExternalInput").ap()
    bqk_ap = nc.dram_tensor("bqk", [P, 2], f32, kind="ExternalInput").ap()
    bvg_ap = nc.dram_tensor("bvg", [P, 8], f32, kind="ExternalInput").ap()
    m8t_ap = nc.dram_tensor("m8t", [P, 32], f32, kind="ExternalInput").ap()
    wp_ap = nc.dram_tensor("wp", [E, E], fmm, kind="ExternalInput").ap()
    bpt_ap = nc.dram_tensor("bpt", [P, 8], f32, kind="ExternalInput").ap()
    e16_ap = nc.dram_tensor("e16", [16, E], fmm, kind="ExternalInput").ap()
    ident_ap = nc.dram_tensor("ident", [P, P], f32, kind="ExternalInput").ap()
    yt_ap = nc.dram_tensor("yt", [E, CH], f32, kind="ExternalOutput").ap()

    with tile.TileContext(nc) as tc:
        with tc.tile_pool(name="const", bufs=1) as constp, \
             tc.tile_pool(name="big", bufs=1) as bigp, \
             tc.tile_pool(name="xbp", bufs=12) as xbp, \
             tc.tile_pool(name="s1w", bufs=2) as s1w, \
             tc.tile_pool(name="qw", bufs=2) as qw, \
             tc.tile_pool(name="esp", bufs=2) as esp, \
             tc.tile_pool(name="otp", bufs=2) as otp, \
             tc.tile_pool(name="s5o", bufs=1) as s5o, \
             tc.tile_pool(name="s5p", bufs=2) as s5p, \
             tc.tile_pool(name="pA", bufs=2, space="PSUM") as pA, \
             tc.tile_pool(name="pB", bufs=2, space="PSUM") as pB, \
             tc.tile_pool(name="pC", bufs=1, space="PSUM") as pC, \
             tc.tile_pool(name="dram", bufs=1, space="DRAM") as dramp:

            bqkt = constp.tile([P, 2], f32)
            nc.sync.dma_start(bqkt[:], bqk_ap)
            bvgt = constp.tile([P, 8], f32)
            nc.sync.dma_start(bvgt[:], bvg_ap)
            bptt = constp.tile([P, 8], f32)
            nc.sync.dma_start(bptt[:], bpt_ap)
            e16t = constp.tile([16, E], fmm)
            nc.sync.dma_start(e16t[:], e16_ap)
            m8tt = constp.tile([P, 32], f32)
            nc.sync.dma_start(m8tt[:], m8t_ap)
            emask = constp.tile([P, 32], fmm, name="emask", tag="emask")
            nc.scalar.activation(emask[:], m8tt[:], EXP)
            identt = constp.tile([P, P], f32)
            nc.sync.dma_start(identt[:], ident_ap)
            wts = []
            for eb in range(NB):
                wt = constp.tile([P, 384], fmm, name=f"wt{eb}", tag=f"wt{eb}")
                nc.sync.dma_start(wt[:], wqkv_ap[eb * P:(eb + 1) * P, :])
                wts.append(wt)
            # c_proj weights are loaded late (only needed after the AllToAll)
            # so the 4MB transfer doesn't delay the first QKV chunks
            wps = []
            for eb in range(NB):
                wpt = constp.tile([P, E], fmm, name=f"wp{eb}", tag=f"wp{eb}")
                wps.append(wpt)

            # persistent feature-major tensors [128 = 2 heads x 64, 4096 tok]
            qT = bigp.tile([P, TOK], fmm, tag="qT")
            kdT = bigp.tile([P, TOK], fmm, tag="kdT")
            vtiles = []   # [gkb][h] -> [128 kpos, 65] (64 feats + emask col)

            agin = dramp.tile([NCH, OSH], fmm, tag="agin")
            agout = dramp.tile([NCH, OSH], fmm, tag="agout")

            def s12(ch):
                t0 = ch * CH
                # ---- S1: Q^T/K^T GEMM (feature-major, bias as K=1 row) ----
                xbs = []
                for eb in range(NB):
                    xb = xbp.tile([P, CH], fmm, name=f"xb{ch}_{eb}", tag="xb")
                    nc.sync.dma_start(
                        xb[:], xt_ap[eb * P:(eb + 1) * P, t0:t0 + CH])
                    xbs.append(xb)
                kc = s1w.tile([P, CH], f32, tag="kc")
                vc = s1w.tile([P, CH], f32, tag="vc")
                for m in range(3):
                    gps = pA.tile([P, CH], f32, tag="pa")
                    for eb in range(NB):
                        nc.tensor.matmul(
                            gps[:], wts[eb][:, m * P:(m + 1) * P], xbs[eb][:],
                            start=(eb == 0), stop=(eb == NB - 1))
                    if m == 2:
                        nc.vector.tensor_copy(vc[:], gps[:])
                    else:
                        dst = qT[:, t0:t0 + CH] if m == 0 else kc[:]
                        nc.vector.tensor_tensor(
                            dst, gps[:],
                            bqkt[:, m:m + 1].to_broadcast((P, CH)), ADD)

                # ---- S2a: KIVI fake-quant of K (DVE + GpSimd) ----
                kq = qw.tile([P, CH], f32, tag="kq")
                nc.vector.transpose(kq[:], kc[:])
                gmax = qw.tile([P, P], f32, tag="gmax")
                nc.vector.tensor_reduce(
                    gmax[:], kq[:].rearrange("p (g f) -> p g f", f=4),
                    axis=X, op=MAX, apply_absolute_value=True)
                rs = qw.tile([P, P], f32, tag="rs")
                nc.vector.reciprocal(rs[:], gmax[:])
                kd = qw.tile([P, CH], f32, tag="kd")
                kd_g = kd[:].rearrange("p (g f) -> p g f", f=4)
                kq_g = kq[:].rearrange("p (g f) -> p g f", f=4)
                nc.gpsimd.tensor_tensor(
                    kd_g, kq_g, rs[:, :, None].to_broadcast((P, P, 4)), MULT)
                # codes = rint(1.5*kd + 1.5) in [0,3]; then codes - 1.5
                nc.vector.tensor_scalar(kd[:], kd[:], 1.5, 1.5, MULT, ADD)
                nc.vector.tensor_scalar(kd[:], kd[:], MAGIC, MAGIC, ADD, SUB)
                nc.vector.tensor_scalar_sub(kd[:], kd[:], 1.5)
                # dequant*1.5: (codes-1.5)*absmax  (1/1.5 folded into W_q)
                kdq = qw.tile([P, CH], f32, tag="kdq")
                nc.gpsimd.tensor_tensor(
                    kdq[:].rearrange("p (g f) -> p g f", f=4), kd_g,
                    gmax[:, :, None].to_broadcast((P, P, 4)), MULT)
                kdn = qw.tile([P, CH], f32, tag="kdn")
                nc.vector.transpose(kdn[:], kdq[:])
                nc.scalar.copy(kdT[:, t0:t0 + CH], kdn[:])

                # ---- S2b: V tiles (transpose to token-major, mask-scaled) --
                for tb in range(4):
                    gkb = ch * 4 + tb
                    psv = pA.tile([P, P], f32, tag="pa")
                    nc.tensor.transpose(
                        psv[:], vc[:, tb * P:(tb + 1) * P], identt[:])
                    vh = []
                    for h in range(2):
                        v = bigp.tile([P, 65], fmm, name=f"v{gkb}_{h}",
                                      tag=f"v{gkb}_{h}")
                        nc.vector.tensor_tensor(
                            v[:, 0:64], psv[:, h * 64:(h + 1) * 64],
                            emask[:, gkb:gkb + 1].to_broadcast((P, 64)), MULT)
                        nc.vector.tensor_copy(
                            v[:, 64:65], emask[:, gkb:gkb + 1])
                        vh.append(v)
                    vtiles.append(vh)

            def s3(b):
                for qc in range(2):
                    j = b * 2 + qc
                    q0 = j * CH
                    cav = [pC.tile([65, CH], f32, name=f"cav{j}_{h}",
                                   tag=f"cav{h}")
                           for h in range(2)]
                    es_list = []

                    def av(kb):
                        gkb = b * 8 + kb
                        for h in range(2):
                            nc.tensor.matmul(
                                cav[h][:], vtiles[gkb][h][:],
                                es_list[kb][:, h * CH:(h + 1) * CH],
                                start=(kb == 0), stop=(kb == 7),
                                skip_group_check=True)

                    for kb in range(8):
                        gkb = b * 8 + kb
                        ps_s = pB.tile([P, 2 * CH], f32, tag="pb")
                        for h in range(2):
                            nc.tensor.matmul(
                                ps_s[:, h * CH:(h + 1) * CH],
                                kdT[h * 64:(h + 1) * 64,
                                    gkb * P:(gkb + 1) * P],
                                qT[h * 64:(h + 1) * 64, q0:q0 + CH],
                                start=True, stop=True)
                        es = esp.tile([P, 2 * CH], fmm, tag="es")
                        nc.scalar.activation(es[:], ps_s[:], EXP, scale=0.125)
                        es_list.append(es)
                        if kb >= 1:
                            av(kb - 1)
                    av(7)
                    for h in range(2):
                        o65 = otp.tile([65, CH], fmm, name=f"o65{j}_{h}",
                                       tag=f"o65{h}")
                        nc.vector.tensor_copy(o65[:], cav[h][:])
                        nc.sync.dma_start(
                            agin[j, h * HSH:(h + 1) * HSH]
                            .rearrange("(p f) -> p f", p=65), o65[:])

            # software-pipelined program order: attention for batch b sits
            # between the QKV/quant of later chunks so PE never waits on DVE
            s12(0); s12(1); s12(2)
            s3(0)
            s12(3); s12(4)
            s3(1)
            for eb in range(NB):
                nc.sync.dma_start(wps[eb][:], wp_ap[eb * P:(eb + 1) * P, :])
            s12(5); s12(6)
            s3(2)
            s12(7)
            s3(3)

            # ---------------- S4: AllToAll ---------------------------
            if sim_single:
                nc.gpsimd.dma_start(agout[:], agin[:])
            else:
                nc.gpsimd.collective_compute(
                    "AllToAll", mybir.AluOpType.bypass,
                    replica_groups=[list(range(8))],
                    ins=[agin[:]], outs=[agout[:]])

            # ---------------- S5: normalize + c_proj (my 512 tokens) --
            rcpin = constp.tile([16, CH], fmm, tag="rcpin")
            nc.sync.dma_start(rcpin[0:8, :], agout[:, 64 * CH:HSH])
            nc.sync.dma_start(rcpin[8:16, :], agout[:, HSH + 64 * CH:OSH])
            rcpt = constp.tile([16, CH], fmm, tag="rcpt")
            with nc.allow_low_precision(reason="tf32 recip"):
                nc.vector.reciprocal(rcpt[:], rcpin[:])
            ogs = []
            for fb in range(NB):
                og = s5o.tile([P, CH], fmm, name=f"og{fb}", tag=f"og{fb}")
                nc.sync.dma_start(
                    og[0:64, :],
                    agout[fb, 0:64 * CH].rearrange("(p f) -> p f", p=64))
                nc.sync.dma_start(
                    og[64:128, :],
                    agout[fb, HSH:HSH + 64 * CH]
                    .rearrange("(p f) -> p f", p=64))
                ps_r = pA.tile([P, CH], f32, tag="pa")
                nc.tensor.matmul(ps_r[:], e16t[:, fb * P:(fb + 1) * P],
                                 rcpt[:], start=True, stop=True)
                rrep = s5p.tile([P, CH], f32, tag="rrep")
                nc.scalar.copy(rrep[:], ps_r[:])
                nc.vector.tensor_tensor(og[:], og[:], rrep[:], MULT)
                # deferred V bias: attn @ (V + 1 bv^T) == o~ + den bv^T, so
                # after normalization just add bv per feature row
                nc.vector.tensor_scalar_add(og[:], og[:], bvgt[:, fb:fb + 1])
                ogs.append(og)
            for ob in range(NB):
                ps_p = pA.tile([P, CH], f32, tag="pa")
                for fb in range(NB):
                    nc.tensor.matmul(
                        ps_p[:], wps[fb][:, ob * P:(ob + 1) * P], ogs[fb][:],
                        start=(fb == 0), stop=(fb == NB - 1))
                yts = s5p.tile([P, CH], f32, tag="yts")
                nc.vector.tensor_tensor(
                    yts[:], ps_p[:],
                    bptt[:, ob:ob + 1].to_broadcast((P, CH)), ADD)
                nc.sync.dma_start(yt_ap[ob * P:(ob + 1) * P, :], yts[:])

    nc.compile()
    return nc


def make_in_maps(hidden_states, attention_mask, w_attn, b_attn, w_proj, b_proj):
    x = np.asarray(hidden_states, np.float32).reshape(TOK, E)
    xt = np.ascontiguousarray(x.T)
    mask = np.asarray(attention_mask, np.float32)
    wa = np.asarray(w_attn, np.float32)
    ba = np.asarray(b_attn, np.float32)
    wpf = np.ascontiguousarray(np.asarray(w_proj, np.float32))
    bp = np.asarray(b_proj, np.float32)

    m8 = (mask * np.float32(0.125)).reshape(4, 8, 128)
    m8t = np.ascontiguousarray(m8.transpose(2, 0, 1).reshape(128, 32))
    ident = np.eye(P, dtype=np.float32)
    bpt = np.ascontiguousarray(bp.reshape(8, P).T)
    e16 = np.zeros((16, E), dtype=np.float32)
    for r in range(16):
        h = 2 * r if r < 8 else 2 * (r - 8) + 1
        fb, half = h // 2, h % 2
        e16[r, fb * P + half * 64: fb * P + half * 64 + 64] = 1.0

    in_maps = []
    for c in range(8):
        cs = slice(c * P, (c + 1) * P)
        wqkv = np.concatenate(
            [wa[:, cs] * np.float32(2.0 / 3.0),
             wa[:, 1024 + c * P:1024 + (c + 1) * P],
             wa[:, 2048 + c * P:2048 + (c + 1) * P]], axis=1)
        bqk = np.stack(
            [ba[cs] * np.float32(2.0 / 3.0),
             ba[1024 + c * P:1024 + (c + 1) * P]], axis=1)
        bvg = np.ascontiguousarray(ba[2048:3072].reshape(8, P).T)
        in_maps.append({
            "xt": xt, "wqkv": np.ascontiguousarray(wqkv),
            "bqk": np.ascontiguousarray(bqk), "bvg": bvg, "m8t": m8t,
            "wp": wpf, "bpt": bpt, "e16": e16, "ident": ident,
        })
    return in_maps


def kernel(hidden_states, attention_mask, w_attn, b_attn, w_proj, b_proj):
    from concourse import bass_utils
    if "nc" not in _CACHE:
        _CACHE["nc"] = _build()
    nc = _CACHE["nc"]
    in_maps = make_in_maps(hidden_states, attention_mask, w_attn, b_attn,
                           w_proj, b_proj)
    res = bass_utils.run_bass_kernel_spmd(nc, in_maps, core_ids=list(range(8)))
    y = np.empty((TOK, E), dtype=np.float32)
    for c in range(8):
        y[c * CH:(c + 1) * CH, :] = res.results[c]["yt"].T
    return y.reshape(4, 1024, E)
